# revision 5
# baseline (speedup 1.0000x reference)
"""Causal self-attention Bass kernel for Trainium2, 8-core data-parallel.

Problem: B=8, S=2048, C=256, H=4 heads, D=64. out = proj(causal_attn(qkv(x))).
Sharding: data-parallel over batch — one batch element per NeuronCore.

Per-core algorithm (all matmuls fp32r = full-rate PE with ~1e-4 rel rounding):
  - transpose x -> xT [c, s] and W -> W.T on PE (needed because the PE
    contracts over the partition dim).
  - qT, kT computed in [d, s] layout directly (out = W_attnT.T @ xT).
  - v computed in [s, d] layout, stored per (s-tile, head) as 128-col blocks
    [v_h (64) | ones (64)]: used as the stationary operand of the attn@v
    matmul, the ones half makes PSUM rows 64..127 accumulate sum(exp) -
    replicated across 64 partitions - for free.
  - scoresT [sk, sq] blocks via lhsT=kT, rhs=qT; heads of a pair live at
    base_partition 0/64 so their K=64 matmuls run concurrently on disjoint
    PE row groups.
  - causal mask for diagonal blocks is ADDED IN PSUM by bf16 mask-matmuls:
    UT(0/1).T @ LT(-1e30) = -1e30*max(p-g,0); fully-masked left sub-blocks
    get a rank-1 ones.T @ (-1e30 row) matmul. exp then gives exact 0.
  - softmax without max-subtraction (|scores/8| < ~3 so exp cannot overflow):
    exp(0.125*scores) on ScalarE straight out of PSUM over 3-bank groups
    (1536 cols/instruction to amortize ACT's 352-cycle fixed cost).
  - attn@v: out.T orientation [d|sum, sq] accumulated over sk tiles in PSUM;
    normalize with DVE reciprocal(rows 64:128) * rows 0:64 -> yT [c_in, s].
  - proj: out = yT.T @ W_projT, written back [s, c].
"""
import numpy as np

import concourse.bass as bass
import concourse.tile as tile
from concourse import bacc, mybir
from concourse.bass_utils import run_bass_kernel_spmd
from concourse.masks import make_identity

dt = mybir.dt
F32 = dt.float32
F32R = dt.float32r
BF16 = dt.bfloat16
AF = mybir.ActivationFunctionType
ALU = mybir.AluOpType

S = 2048
C = 256
H = 4
D = 64
B = 8
ST = S // 128            # 16 s-tiles
SB = S // 512            # 4 sq-blocks of 512
NEG = -1e30
GROUP = 3                # sk-slots per exp group (3 PSUM banks)


def _emit(nc, tc, ctx, x, wa, wp, out):
    import contextlib

    const = ctx.enter_context(tc.tile_pool(name="const", bufs=1))
    per = ctx.enter_context(tc.tile_pool(name="persist", bufs=1))

    # ---- constants ----
    ident = const.tile([128, 128], F32, tag="ident")
    make_identity(nc, ident[:])
    ut_bf = const.tile([128, 128], BF16, tag="ut")       # ut[m,p] = 1 if p>=m
    nc.gpsimd.memset(ut_bf[:], 1.0)
    nc.gpsimd.affine_select(out=ut_bf[:], in_=ut_bf[:], compare_op=ALU.is_ge,
                            fill=0.0, base=0, pattern=[[1, 128]], channel_multiplier=-1)
    lt_bf = const.tile([128, 128], BF16, tag="lt")       # lt[m,g] = NEG if m>g
    nc.gpsimd.memset(lt_bf[:], NEG)
    nc.gpsimd.affine_select(out=lt_bf[:], in_=lt_bf[:], compare_op=ALU.is_gt,
                            fill=0.0, base=0, pattern=[[-1, 128]], channel_multiplier=1)
    ones_k1 = const.tile([1, 128], BF16, tag="ones_k1")  # rank-1 mask lhsT
    nc.gpsimd.memset(ones_k1[:], 1.0)
    neg_row = const.tile([1, 384], BF16, tag="neg_row")  # rank-1 mask rhs
    nc.gpsimd.memset(neg_row[:], NEG)

    # ---- persistent SBUF tensors ----
    xT = [per.tile([128, S], F32R, tag=f"xT{ci}", name=f"xT{ci}") for ci in range(2)]
    wT = [per.tile([128, 768], F32R, tag=f"wT{ci}", name=f"wT{ci}") for ci in range(2)]   # W_attn.T
    wpT = [per.tile([128, 256], F32R, tag=f"wpT{ci}", name=f"wpT{ci}") for ci in range(2)]  # W_proj.T
    qkT = [per.tile([128, S], F32R, tag=f"qkT{ob}", name=f"qkT{ob}") for ob in range(4)]
    # v blocks: [128, st, h, (v|ones)] = [128, 16*4*128]
    v_sb = per.tile([128, ST * H * 128], F32R, tag="v")
    yT = [per.tile([128, S], F32R, tag=f"yT{ci}", name=f"yT{ci}") for ci in range(2)]

    v4 = v_sb[:].rearrange("p (t h x) -> p t h x", t=ST, h=H)
    v3 = v_sb[:].bitcast(F32).rearrange("p (c x) -> p c x", x=128)
    nc.gpsimd.memset(v3[:, :, 64:128], 1.0)

    # ================= phase 1: transposes + qkv =================
    with contextlib.ExitStack() as ph1:
        io_pool = ph1.enter_context(tc.tile_pool(name="io", bufs=4))
        ps_a = ph1.enter_context(tc.tile_pool(name="ps_a", bufs=2, space="PSUM"))
        ps_b = ph1.enter_context(tc.tile_pool(name="ps_b", bufs=2, space="PSUM"))

        # W_attn.T: 6 o-blocks x 2 c-blocks
        for ob in range(6):
            w_nat = io_pool.tile([128, 256], F32, tag="w_nat")
            nc.sync.dma_start(w_nat[:], wa[ob * 128:(ob + 1) * 128, :])
            for ci in range(2):
                tp = ps_b.tile([128, 128], F32, tag="tp_w")
                nc.tensor.transpose(tp[:], w_nat[:, ci * 128:(ci + 1) * 128], ident[:])
                nc.vector.tensor_copy(wT[ci][:, ob * 128:(ob + 1) * 128], tp[:])
        # W_proj.T
        for ob in range(2):
            w_nat = io_pool.tile([128, 256], F32, tag="w_nat")
            nc.sync.dma_start(w_nat[:], wp[ob * 128:(ob + 1) * 128, :])
            for ci in range(2):
                tp = ps_b.tile([128, 128], F32, tag="tp_w")
                nc.tensor.transpose(tp[:], w_nat[:, ci * 128:(ci + 1) * 128], ident[:])
                nc.vector.tensor_copy(wpT[ci][:, ob * 128:(ob + 1) * 128], tp[:])

        # x -> xT ([c, s]); transpose 4 consecutive s-tiles into one psum bank group
        for ci in range(2):
            for sg in range(4):          # groups of 4 s-tiles
                tp = ps_a.tile([128, 512], F32, tag="tp_x")
                for k in range(4):
                    st = sg * 4 + k
                    x_nat = io_pool.tile([128, 256], F32, tag=f"x_nat{ci}")
                    nc.sync.dma_start(x_nat[:], x[st * 128:(st + 1) * 128, :])
                    nc.tensor.transpose(tp[:, k * 128:(k + 1) * 128],
                                        x_nat[:, ci * 128:(ci + 1) * 128], ident[:])
                nc.vector.tensor_copy(xT[ci][:, sg * 512:(sg + 1) * 512], tp[:])

        # qT/kT: 4 o-blocks (q01, q23, k01, k23) x 4 s-blocks
        for ob in range(4):
            for sb in range(SB):
                ps = ps_a.tile([128, 512], F32, tag="ps_qk")
                for ci in range(2):
                    nc.tensor.matmul(ps[:], wT[ci][:, ob * 128:(ob + 1) * 128],
                                     xT[ci][:, sb * 512:(sb + 1) * 512],
                                     start=(ci == 0), stop=(ci == 1))
                nc.vector.tensor_copy(qkT[ob][:, sb * 512:(sb + 1) * 512], ps[:])

        # v: [s, o_v] per s-tile, strided into (v|ones) blocks
        for st in range(ST):
            ps = ps_b.tile([128, 256], F32, tag="ps_v")
            for ci in range(2):
                nc.tensor.matmul(ps[:], xT[ci][:, st * 128:(st + 1) * 128],
                                 wT[ci][:, 512:768],
                                 start=(ci == 0), stop=(ci == 1))
            nc.vector.tensor_copy(v4[:, st, :, 0:64],
                                  ps[:].rearrange("p (h d) -> p h d", h=H))

    # ================= phase 2: attention =================
    with contextlib.ExitStack() as ph2:
        ps_g = ph2.enter_context(tc.tile_pool(name="ps_g", bufs=2, space="PSUM"))
        ps_o = ph2.enter_context(tc.tile_pool(name="ps_o", bufs=1, space="PSUM"))
        ex_pool = ph2.enter_context(tc.tile_pool(name="expT", bufs=2))
        rc_pool = ph2.enter_context(tc.tile_pool(name="rc", bufs=2))

        for pair in range(2):            # heads (0,1) then (2,3)
            qTp, kTp = qkT[pair], qkT[2 + pair]
            for b in range(SB):
                nt = 4 * b + 4           # sk-tiles for this sq-block
                slots = [(t, h) for t in range(nt) for h in range(2)]
                O = [ps_o.tile([128, 512], F32, tag=f"O{_h}", name=f"O{_h}") for _h in range(2)]
                sq = slice(b * 512, (b + 1) * 512)

                for g0 in range(0, len(slots), GROUP):
                    chunk = slots[g0:g0 + GROUP]
                    G = ps_g.tile([128, GROUP * 512], F32, tag="G")
                    ex = ex_pool.tile([128, GROUP * 512], F32R, tag="ex")
                    # scores (+ causal masks on diagonal tiles)
                    for i, (t, h) in enumerate(chunk):
                        hh = slice(h * 64, h * 64 + 64)
                        gcol = slice(i * 512, (i + 1) * 512)
                        j = t - 4 * b    # >=0 on diagonal tiles
                        nc.tensor.matmul(G[:, gcol], kTp[hh, t * 128:(t + 1) * 128],
                                         qTp[hh, sq], start=True, stop=(j < 0),
                                         skip_group_check=True)
                        if j >= 0:
                            nc.tensor.matmul(G[:, i * 512 + j * 128: i * 512 + (j + 1) * 128],
                                             ut_bf[:], lt_bf[:], start=False, stop=(j == 0),
                                             skip_group_check=True)
                            if j > 0:
                                nc.tensor.matmul(G[:, i * 512: i * 512 + j * 128],
                                                 ones_k1[:], neg_row[0:1, 0:j * 128],
                                                 start=False, stop=True,
                                                 skip_group_check=True)
                    # exp of the whole group straight out of PSUM
                    w = len(chunk) * 512
                    nc.scalar.activation(ex[:, 0:w], G[:, 0:w], AF.Exp, scale=0.125)
                    # attn @ v (+ running sum via the ones half)
                    for i, (t, h) in enumerate(chunk):
                        nc.tensor.matmul(O[h][:], v4[:, t, pair * 2 + h, :],
                                         ex[:, i * 512:(i + 1) * 512],
                                         start=(t == 0), stop=(t == nt - 1),
                                         skip_group_check=True)

                for h in range(2):
                    rc = rc_pool.tile([64, 512], F32, tag="rc")
                    nc.vector.reciprocal(rc[:], O[h][64:128, :])
                    nc.vector.tensor_tensor(yT[pair][h * 64:(h + 1) * 64, sq],
                                            O[h][0:64, :], rc[:], ALU.mult)

    # ================= phase 3: projection =================
    with contextlib.ExitStack() as ph3:
        ps_p = ph3.enter_context(tc.tile_pool(name="ps_p", bufs=2, space="PSUM"))
        out_pool = ph3.enter_context(tc.tile_pool(name="out_sb", bufs=4))
        for st in range(ST):
            ps = ps_p.tile([128, 256], F32, tag="ps_p")
            for ci in range(2):
                nc.tensor.matmul(ps[:], yT[ci][:, st * 128:(st + 1) * 128],
                                 wpT[ci][:], start=(ci == 0), stop=(ci == 1))
            o_sb = out_pool.tile([128, 256], F32, tag="o_sb")
            nc.vector.tensor_copy(o_sb[:], ps[:])
            nc.sync.dma_start(out[st * 128:(st + 1) * 128, :], o_sb[:])


_CACHE = {}


def _build():
    if "nc" in _CACHE:
        return _CACHE["nc"]
    from contextlib import ExitStack

    nc = bacc.Bacc("TRN2", target_bir_lowering=False, debug=False)
    x = nc.dram_tensor("x", [S, C], F32, kind="ExternalInput").ap()
    wa = nc.dram_tensor("w_attn", [3 * C, C], F32, kind="ExternalInput").ap()
    wp = nc.dram_tensor("w_proj", [C, C], F32, kind="ExternalInput").ap()
    out = nc.dram_tensor("out", [S, C], F32, kind="ExternalOutput").ap()
    with tile.TileContext(nc) as tc, ExitStack() as ctx:
        _emit(nc, tc, ctx, x, wa, wp, out)
    nc.compile()
    _CACHE["nc"] = nc
    return nc


def kernel(x, W_attn, W_proj):
    x = np.ascontiguousarray(np.asarray(x, dtype=np.float32))
    W_attn = np.ascontiguousarray(np.asarray(W_attn, dtype=np.float32))
    W_proj = np.ascontiguousarray(np.asarray(W_proj, dtype=np.float32))
    nc = _build()
    in_maps = [{"x": x[b], "w_attn": W_attn, "w_proj": W_proj} for b in range(B)]
    res = run_bass_kernel_spmd(nc, in_maps, core_ids=list(range(B)))
    return np.stack([res.results[b]["out"] for b in range(B)], axis=0)


# revision 12
# speedup vs baseline: 39.9777x; 39.9777x over previous
"""Causal self-attention Bass kernel for Trainium2, 8-core data-parallel.

Problem: B=8, S=2048, C=256, H=4 heads, D=64. out = proj(causal_attn(qkv(x))).
Sharding: data-parallel over batch - one batch element per NeuronCore.

Per-core design (all matmuls fp32r = full-rate PE, ~1e-4 rel rounding):
  - PE contracts over the partition dim: weights are transposed on the host
    (free numpy repack in kernel()); x is transposed on-chip via PE
    transpose-mode, grouped through shared PSUM tiles.
  - qT, kT in [d, s] layout ([128, 2048] per head-pair: head0 rows 0..63,
    head1 rows 64..127 - the two heads' K=64 score matmuls land on disjoint
    PE row groups and run concurrently).
  - v stored per (s-tile, head) as [v_h (64) | ones (64)] 128-col stationary
    blocks: the ones half makes attn@v emit sum(exp) replicated across PSUM
    rows 64..127 for free.
  - scoresT [sk, sq] blocks, softmax WITHOUT max subtraction (|scores|/8 is
    small): exp(0.125*x) on ScalarE straight out of PSUM, over multi-slot
    groups to amortize ACT's 352-cycle fixed cost.
  - causal handling: interior sk-tiles full-width; diagonal sk-tiles compute
    only the valid column range, with the triangular boundary masked IN PSUM
    by a bf16 mask-matmul (UT(0/1).T @ LT(-1e30) = -1e30*max(p-g,0)).
  - attn@v accumulates outT [d|sum, sq] over sk-tiles in PSUM; normalization
    = sums copied to a base-partition-0 tile, reciprocal_approx_fast, then
    one DVE multiply -> yT [c_in, s].
  - proj: out = yT.T @ W_projT -> [s, c] -> DMA out.
  - emission is driven as a single unit stream with a depth-2 software
    pipeline skew (attn@v trails scores/exp by 2 units, across block
    boundaries) plus interleaved x-transpose/qkv/proj prefetch items.
"""
import numpy as np

import concourse.bass as bass
import concourse.tile as tile
from concourse import bacc, mybir
from concourse.bass_utils import run_bass_kernel_spmd
from concourse.masks import make_identity

dt = mybir.dt
F32 = dt.float32
F32R = dt.float32r
BF16 = dt.bfloat16
AF = mybir.ActivationFunctionType
ALU = mybir.AluOpType

S = 2048
C = 256
H = 4
D = 64
B = 8
ST = S // 128            # 16 s-tiles
SB = S // 512            # 4 sq-blocks of 512
NEG = -1e30
GROUP = 3                # interior slots per exp group (3 PSUM banks)


def _emit(nc, tc, ctx, x, wa, wp, out, dbg=None):
    const = ctx.enter_context(tc.tile_pool(name="const", bufs=1))
    per = ctx.enter_context(tc.tile_pool(name="persist", bufs=1))
    # unified PSUM pools for the whole kernel: 2*3 + 1 + 1 = 8 banks
    ps_g = ctx.enter_context(tc.tile_pool(name="ps_g", bufs=2, space="PSUM"))
    ps_o = ctx.enter_context(tc.tile_pool(name="ps_o", bufs=1, space="PSUM"))
    io_pool = ctx.enter_context(tc.tile_pool(name="io", bufs=8))
    ex_pool = ctx.enter_context(tc.tile_pool(name="expT", bufs=4))
    rc_pool = ctx.enter_context(tc.tile_pool(name="rc", bufs=3))
    out_pool = ctx.enter_context(tc.tile_pool(name="out_sb", bufs=2))

    def gtile(name):
        return ps_g.tile([128, GROUP * 512], F32, tag="G", name=name)

    def otile(h):
        return ps_o.tile([128, 512], F32, tag=f"O{h}", name=f"O{h}")

    def copy_split(dst, src, w):
        """Evacuate [128, w] PSUM->SBUF using DVE and ACT halves in parallel."""
        half = (w // 2 + 127) & ~127
        nc.vector.tensor_copy(dst[:, 0:half], src[:, 0:half])
        nc.scalar.copy(dst[:, half:w], src[:, half:w])

    # ---- constants ----
    ident = const.tile([128, 128], F32, tag="ident")
    make_identity(nc, ident[:])
    ut_bf = const.tile([128, 128], BF16, tag="ut")       # ut[m,p] = 1 if p>=m
    nc.gpsimd.memset(ut_bf[:], 1.0)
    nc.gpsimd.affine_select(out=ut_bf[:], in_=ut_bf[:], compare_op=ALU.is_ge,
                            fill=0.0, base=0, pattern=[[1, 128]], channel_multiplier=-1)
    lt_bf = const.tile([128, 128], BF16, tag="lt")       # lt[m,g] = NEG if m>g
    nc.gpsimd.memset(lt_bf[:], NEG)
    nc.gpsimd.affine_select(out=lt_bf[:], in_=lt_bf[:], compare_op=ALU.is_gt,
                            fill=0.0, base=0, pattern=[[-1, 128]], channel_multiplier=1)

    # ---- persistent SBUF tensors ----
    xT = [per.tile([128, S], F32R, tag=f"xT{ci}", name=f"xT{ci}") for ci in range(2)]
    wT = [per.tile([128, 768], F32R, tag=f"wT{ci}", name=f"wT{ci}") for ci in range(2)]
    wpT = [per.tile([128, 256], F32R, tag=f"wpT{ci}", name=f"wpT{ci}") for ci in range(2)]
    qkT = [per.tile([128, S], F32R, tag=f"qkT{ob}", name=f"qkT{ob}") for ob in range(4)]
    v_sb = per.tile([128, ST * H * 128], F32R, tag="v")
    yT = [per.tile([128, S], F32R, tag=f"yT{ci}", name=f"yT{ci}") for ci in range(2)]

    v4 = v_sb[:].rearrange("p (t h x) -> p t h x", t=ST, h=H)
    v3m = v_sb[:].bitcast(F32).rearrange("p (c x) -> p c x", x=128)
    nc.gpsimd.memset(v3m[:, :, 64:128], 1.0)

    # ================= phase 1 =================
    # warm the exp table while phase 1 runs
    dummy = const.tile([1, 8], F32, tag="dummy")
    nc.scalar.activation(dummy[:], ident[0:1, 0:8], AF.Exp, scale=1.0)

    # x -> xT groups; group 0 covers exactly what attention b=0 needs
    xgroups = [(0, 4), (4, 6), (10, 6)]

    def emit_xg(gi):
        st0, nst = xgroups[gi]
        x_nat = io_pool.tile([128, nst * 256], F32, tag="x_nat", bufs=2,
                             name=f"x_nat{gi}")
        nc.sync.dma_start(
            x_nat[:].rearrange("p (k c) -> p k c", k=nst),
            x[st0 * 128:(st0 + nst) * 128, :].rearrange("(k p) c -> p k c", p=128))
        nats = [x_nat[:, k * 256:(k + 1) * 256] for k in range(nst)]
        for ci in range(2):
            xg = gtile(f"xg{gi}_{ci}")
            for k in range(nst):
                nc.tensor.transpose(xg[:, k * 128:(k + 1) * 128],
                                    nats[k][:, ci * 128:ci * 128 + 128], ident[:])
            copy_split(xT[ci][:, st0 * 128:(st0 + nst) * 128], xg, nst * 128)
        if gi == 0:
            for ci in range(2):
                nc.sync.dma_start(wT[ci][:],
                                  wa[ci * 128:(ci + 1) * 128, :].bitcast(F32R))

    # qkv for one sq-block, split into two stream items
    def emit_qkv_a(sb):
        qg = gtile(f"qg{sb}")          # obs 0..2
        for ob in range(3):
            for ci in range(2):
                nc.tensor.matmul(qg[:, ob * 512:(ob + 1) * 512],
                                 wT[ci][:, ob * 128:(ob + 1) * 128],
                                 xT[ci][:, sb * 512:(sb + 1) * 512],
                                 start=(ci == 0), stop=(ci == 1))
        for ob in range(3):
            cp = nc.scalar.copy if ob % 2 else nc.vector.tensor_copy
            cp(qkT[ob][:, sb * 512:(sb + 1) * 512],
               qg[:, ob * 512:(ob + 1) * 512])

    def emit_qkv_b(sb):
        # v for the 4 s-tiles of this sb (+ qk ob3 in the spare 512 cols)
        vg = gtile(f"vg{sb}")
        for k in range(4):
            st = sb * 4 + k
            for ci in range(2):
                nc.tensor.matmul(vg[:, k * 256:(k + 1) * 256],
                                 xT[ci][:, st * 128:(st + 1) * 128],
                                 wT[ci][:, 512:768],
                                 start=(ci == 0), stop=(ci == 1))
        for ci in range(2):
            nc.tensor.matmul(vg[:, 1024:1536], wT[ci][:, 384:512],
                             xT[ci][:, sb * 512:(sb + 1) * 512],
                             start=(ci == 0), stop=(ci == 1))
        nc.scalar.copy(qkT[3][:, sb * 512:(sb + 1) * 512], vg[:, 1024:1536])
        for k in range(4):
            st = sb * 4 + k
            nc.vector.tensor_copy(v4[:, st, :, 0:64],
                                  vg[:, k * 256:(k + 1) * 256]
                                  .rearrange("p (h d) -> p h d", h=H))

    def emit_qkv(sb):
        emit_qkv_a(sb)
        emit_qkv_b(sb)

    def attention_units(b, pair):
        """Yield (pre_fn, post_fn) work units for one (sq-block, head-pair).

        pre = scores matmuls + exp; post = attn@v (+ normalize on the last
        unit). The driver runs post one unit behind pre, across block
        boundaries, so the scores/exp pipeline never drains.
        """
        qTp, kTp = qkT[pair], qkT[2 + pair]
        nt = 4 * b + 4               # sk-tiles for this sq-block
        O = [None, None]             # allocated in the first post (attn@v)
        sq = slice(b * 512, (b + 1) * 512)

        units = []
        inner = [(t, h) for t in range(4 * b) for h in range(2)]
        for g0 in range(0, len(inner), GROUP):
            units.append(("int", inner[g0:g0 + GROUP]))
        # (j, h) -> packed col offset; bank-aligned, unit A spans 1024,
        # unit B exactly fills 1536 with no 512-boundary crossings
        units.append(("diag", [(0, 0, 0), (0, 1, 512)]))
        units.append(("diag", [(1, 0, 0), (3, 1, 384), (1, 1, 512),
                               (3, 0, 896), (2, 0, 1024), (2, 1, 1280)]))

        def normalize():
            for h in range(2):
                # sums must land at base-partition 0: reciprocal_approx_fast
                # (custom DVE op) misbehaves on partition-offset inputs
                sm = rc_pool.tile([64, 512], F32, tag="sm", name="sm")
                nc.vector.tensor_copy(sm[:], O[h][64:128, :])
                rc = rc_pool.tile([64, 512], F32, tag="rc", name="rc")
                nc.vector.reciprocal_approx_fast(rc[:], sm[:])
                nc.vector.tensor_tensor(yT[pair][h * 64:(h + 1) * 64, sq],
                                        O[h][0:64, :], rc[:], ALU.mult)

        for ui, (kind, payload) in enumerate(units):
            last = ui == len(units) - 1

            def pre(kind=kind, payload=payload):
                G = gtile("Ga")
                ex = ex_pool.tile([128, GROUP * 512], F32R, tag="ex", name="ex")
                av = []
                if kind == "int":
                    for i, (t, h) in enumerate(payload):
                        hh = slice(h * 64, h * 64 + 64)
                        nc.tensor.matmul(G[:, i * 512:(i + 1) * 512],
                                         kTp[hh, t * 128:(t + 1) * 128],
                                         qTp[hh, sq], start=True, stop=True)
                        av.append((h, slice(0, 512),
                                   ex[:, i * 512:(i + 1) * 512], t))
                    w = len(payload) * 512
                    nc.scalar.activation(ex[:, 0:w], G[:, 0:w], AF.Exp,
                                         scale=0.125)
                else:
                    ext = 0
                    for j, h, c0 in payload:
                        t = 4 * b + j
                        off, w = j * 128, 512 - j * 128   # valid width
                        ext = max(ext, c0 + w)
                        hh = slice(h * 64, h * 64 + 64)
                        nc.tensor.matmul(G[:, c0:c0 + w],
                                         kTp[hh, t * 128:(t + 1) * 128],
                                         qTp[hh, b * 512 + off:(b + 1) * 512],
                                         start=True, stop=False,
                                         skip_group_check=True)
                        nc.tensor.matmul(G[:, c0:c0 + 128], ut_bf[:], lt_bf[:],
                                         start=False, stop=True,
                                         skip_group_check=True)
                        av.append((h, slice(off, 512), ex[:, c0:c0 + w], t))
                    nc.scalar.activation(ex[:, 0:ext], G[:, 0:ext], AF.Exp,
                                         scale=0.125)
                return av

            def post(av, last=last, first=(ui == 0)):
                if first:
                    O[0], O[1] = otile(0), otile(1)
                for h, osl, exsl, t in av:
                    nc.tensor.matmul(O[h][:, osl], v4[:, t, pair * 2 + h, :],
                                     exsl, start=(t == 0), stop=(t == nt - 1),
                                     skip_group_check=True)
                if last:
                    normalize()

            yield pre, post

    def emit_proj(b):
        if b == 0:
            for ci in range(2):
                nc.sync.dma_start(wpT[ci][:],
                                  wp[ci * 128:(ci + 1) * 128, :].bitcast(F32R))
        pg = gtile(f"pg{b}")
        for k in range(4):
            st = b * 4 + k
            for ci in range(2):
                nc.tensor.matmul(pg[:, k * 256:(k + 1) * 256],
                                 yT[ci][:, st * 128:(st + 1) * 128],
                                 wpT[ci][:], start=(ci == 0), stop=(ci == 1))
        o_sb = out_pool.tile([128, 4 * 256], F32, tag="o_sb", bufs=2)
        nc.vector.tensor_copy(o_sb[:], pg[:, 0:1024])
        nc.sync.dma_start(
            out[b * 512:(b + 1) * 512, :].rearrange("(k p) c -> p k c", p=128),
            o_sb[:].rearrange("p (k c) -> p k c", k=4))

    # drive: global one-unit skew over [attention | qkv | proj] streams.
    # proj(b) is placed one attention-unit into block b+1 so that b's last
    # normalize has already been emitted by the skewed driver.
    emit_xg(0)
    stream = [("qkv", 0)]
    for sb in range(SB):
        units0 = [("attn", u) for u in attention_units(sb, 0)]
        stream.extend(units0)
        if sb + 1 < SB:
            stream.append(("qkvb", sb + 1))
        units1 = [("attn", u) for u in attention_units(sb, 1)]
        stream.extend(units1)
        if sb + 1 < SB:
            stream.insert(len(stream) - len(units1) + 1, ("qkva", sb + 1))
        if sb > 0:
            stream.insert(len(stream) - len(units1) + 2, ("proj", sb - 1))
        if sb < 2:  # xg(1) inside b=0 pair0; xg(2) inside b=1 pair0
            stream.insert(len(stream) - len(units1) - len(units0), ("xg", sb + 1))

    from collections import deque
    pending = deque()
    DEPTH = 2
    emitters = {"qkv": emit_qkv, "qkva": emit_qkv_a, "qkvb": emit_qkv_b,
                "xg": emit_xg, "proj": emit_proj}
    for kind, item in stream:
        if kind != "attn":
            emitters[kind](item)
            continue
        pre, post = item
        av = pre()
        if len(pending) >= DEPTH:
            av2, post2 = pending.popleft()
            post2(av2)
        pending.append((av, post))
    while pending:
        av2, post2 = pending.popleft()
        post2(av2)
    emit_proj(SB - 1)

    if dbg is not None:
        for ob in range(4):
            nc.sync.dma_start(dbg[f"qkT{ob}"], qkT[ob][:].bitcast(F32))
        nc.sync.dma_start(dbg["v"], v_sb[:].bitcast(F32))
        for ci in range(2):
            nc.sync.dma_start(dbg[f"yT{ci}"], yT[ci][:].bitcast(F32))



_CACHE = {}


def _build(debug=False):
    key = "nc_dbg" if debug else "nc"
    if key in _CACHE:
        return _CACHE[key]
    from contextlib import ExitStack

    nc = bacc.Bacc("TRN2", target_bir_lowering=False, debug=False)
    x = nc.dram_tensor("x", [S, C], F32, kind="ExternalInput").ap()
    wa = nc.dram_tensor("w_attn_t", [C, 3 * C], F32, kind="ExternalInput").ap()
    wp = nc.dram_tensor("w_proj_t", [C, C], F32, kind="ExternalInput").ap()
    out = nc.dram_tensor("out", [S, C], F32, kind="ExternalOutput").ap()
    dbg = None
    if debug:
        dbg = {}
        for ob in range(4):
            dbg[f"qkT{ob}"] = nc.dram_tensor(f"qkT{ob}", [128, S], F32, kind="ExternalOutput").ap()
        dbg["v"] = nc.dram_tensor("v", [128, ST * H * 128], F32, kind="ExternalOutput").ap()
        for ci in range(2):
            dbg[f"yT{ci}"] = nc.dram_tensor(f"yT{ci}", [128, S], F32, kind="ExternalOutput").ap()
    with tile.TileContext(nc) as tc, ExitStack() as ctx:
        _emit(nc, tc, ctx, x, wa, wp, out, dbg)
    nc.compile()
    _CACHE[key] = nc
    return nc


def kernel(x, W_attn, W_proj):
    x = np.ascontiguousarray(np.asarray(x, dtype=np.float32))
    W_attn = np.ascontiguousarray(np.asarray(W_attn, dtype=np.float32))
    W_proj = np.ascontiguousarray(np.asarray(W_proj, dtype=np.float32))
    nc = _build()
    wat = np.ascontiguousarray(W_attn.T)
    wpt = np.ascontiguousarray(W_proj.T)
    in_maps = [{"x": x[b], "w_attn_t": wat, "w_proj_t": wpt} for b in range(B)]
    res = run_bass_kernel_spmd(nc, in_maps, core_ids=list(range(B)))
    return np.stack([res.results[b]["out"] for b in range(B)], axis=0)

